# revision 14
# baseline (speedup 1.0000x reference)
"""Trainium2 Bass kernel for nn_Attention_45569603010584 — v2.

Per-node causal conv attention + FFN over (B=32, C=64, N=207, T=96).
Shards batch b across 8 cores (4 b's each); xin/yout stay in the
natural (b, c, n*T+t) layout so the host does zero reformatting.

Device-side structure (per core; 8 units of half-a-b each):
  x is DMA'd into a gap-padded SBUF layout [65, 2 + nn*98]: each bn's
  96 tokens sit at stride 98 with 2 zero columns before them, so the
  causal-conv taps are plain column-shifted matmul reads (no shifted
  copies, no SBUF->SBUF DMA). Row 64 is constant 1.0 (gives the
  softmax denominator column through the vT matmul for free).

  Phase A (batched):  q|k = 3 accumulated tap matmuls per 512-col PSUM
                      chunk, split to q/k tiles (base partition 0).
  Phase B (5 bns/chunk):  per bn: vT33 = x_bn^T @ [vw|e65]   [96,33]
                          attnT  = k_bn^T q_bn               [96,96]
                          exp (ACT) -> causal mask (gpsimd mul)
                          aoT    = vT33^T @ E                [33,96]
                      then per chunk: o-proj matmul, reciprocal of the
                      denominator row, PE broadcast of 1/denom to 64
                      partitions (K=1 matmul), normalize-mul, residual
                      add into x. Attention operands are bf16.
  Phase C (batched):  FFN per 512-col chunk: ff1 -> relu(+b1) -> ff2
                      -> add into x (in place); one strided DMA out.
"""

import numpy as np

B, C, N, T = 32, 64, 207, 96
H = 32
NCORES = 8
BPC = B // NCORES        # 4 b's per core
NT = N * T               # 19872
PW = T + 2               # 98: padded token stride
NN1, NN2 = 104, 103      # half-b split of N
PADC_MAX = 2 + NN1 * PW  # 10194
GB = 5                   # bns per attention chunk
ACH = 512                # phase A/C PSUM chunk cols

_CACHE = {}


def _make_tile_context_cls():
    import concourse.mybir as mybir
    from concourse.tile import TileContext, ScopedClock

    class PatchedTileContext(TileContext):
        """The walrus build here rejects instructions carrying more than
        ~2 semaphore waits ("Too many sync wait commands"); TileContext's
        kernel-tail drain aggregates one wait per logical processor onto a
        single Drain. Split them one-per-nop instead."""

        def _split_excess_waits(self):
            nsplit = 0
            for f in self.nc.m.functions:
                for bb in f.blocks:
                    il = bb.instructions
                    out = []
                    for inst in il:
                        si = inst.sync_info
                        if si is not None and len(si.on_wait) > 1:
                            waits = list(si.on_wait)
                            for i, w in enumerate(waits[:-1]):
                                nop = mybir.InstNoOp(
                                    name=f"{inst.name}_wsplit{i}",
                                    engine=inst.engine)
                                nop.sync_info = mybir.SyncInfo(
                                    on_wait=[w], on_update=[])
                                out.append(nop)
                                nsplit += 1
                            inst.sync_info = mybir.SyncInfo(
                                on_wait=waits[-1:],
                                on_update=list(si.on_update))
                        out.append(inst)
                    il[:] = out
            return nsplit

        def _drain_and_barrier(self, tick_clock, wait_clock):
            carrier = self.nc.sync.nop()
            wait_clock.add_sem_waits(
                carrier.ins, ScopedClock({None: tick_clock.global_clock}))
            si = carrier.ins.sync_info
            waits = list(si.on_wait) if si is not None else []
            upd = list(si.on_update) if si is not None else []
            carrier.ins.sync_info = mybir.SyncInfo(on_wait=waits[:1],
                                                   on_update=upd)
            for i in range(1, len(waits)):
                n2 = self.nc.sync.nop()
                n2.ins.sync_info = mybir.SyncInfo(on_wait=waits[i:i + 1],
                                                  on_update=[])
            self.nc.sync.drain()
            self.nc.all_engine_barrier()
            assert self.sems is not None
            popped = self.nc._tile_sem_poison_stack.pop()
            assert popped is self._sem_poison
            self.nc.clear_and_free_semaphores(
                list(self.sems.allocated().values()))
            self.nc.all_engine_barrier()
            self._split_excess_waits()

    return PatchedTileContext


def _build_program(repeat=1):
    import concourse.bass as bass
    import concourse.mybir as mybir
    from contextlib import ExitStack

    TileContext = _make_tile_context_cls()
    FP = mybir.dt.float32
    FR = mybir.dt.float32r
    BF = mybir.dt.bfloat16
    ACT = mybir.ActivationFunctionType
    nc = bass.Bass()

    xin = nc.dram_tensor("xin", [BPC * C, NT], FR, kind="ExternalInput")
    wt_d = nc.dram_tensor("wt", [3 * C, 2 * H], FR, kind="ExternalInput")
    vwt_d = nc.dram_tensor("vwt", [C + 1, 2 * H], FR, kind="ExternalInput")
    owt_d = nc.dram_tensor("owt", [H, C], BF, kind="ExternalInput")
    ff1t_d = nc.dram_tensor("ff1t", [C, C], FR, kind="ExternalInput")
    ff2t_d = nc.dram_tensor("ff2t", [C, C], BF, kind="ExternalInput")
    b1_d = nc.dram_tensor("b1", [C, 1], FP, kind="ExternalInput")
    mask_d = nc.dram_tensor("mask", [T, GB * T], BF, kind="ExternalInput")
    ones_d = nc.dram_tensor("ones1", [1, C], BF, kind="ExternalInput")
    yout = nc.dram_tensor("yout", [BPC * C, NT], FP, kind="ExternalOutput")

    with TileContext(nc) as tc, ExitStack() as ctx:
        const = ctx.enter_context(tc.tile_pool(name="const", bufs=1))

        def load_const(dram, shape, tag, dt=None):
            t = const.tile(shape, dt or FR, tag=tag)
            nc.sync.dma_start(out=t[:], in_=dram[:])
            return t

        wt2_sb = const.tile([C, 2 * H], FR, tag="wt2")
        nc.sync.dma_start(out=wt2_sb[:], in_=wt_d[0:C, :])
        wt1_sb = const.tile([C, 2 * H], FR, tag="wt1")
        nc.sync.dma_start(out=wt1_sb[:], in_=wt_d[C:2 * C, :])
        wt0_sb = const.tile([C, 2 * H], FR, tag="wt0")
        nc.sync.dma_start(out=wt0_sb[:], in_=wt_d[2 * C:3 * C, :])
        vwt_sb = load_const(vwt_d, [C + 1, 2 * H], "vwt")
        owt_sb = load_const(owt_d, [H, C], "owt", dt=BF)
        ff1t_sb = load_const(ff1t_d, [C, C], "ff1t")
        ff2t_sb = load_const(ff2t_d, [C, C], "ff2t", dt=BF)
        b1_sb = load_const(b1_d, [C, 1], "b1", dt=FP)
        mask_sb = load_const(mask_d, [T, GB * T], "mask", dt=BF)
        ones_sb = load_const(ones_d, [1, C], "ones1", dt=BF)

        xp = ctx.enter_context(tc.tile_pool(name="xp", bufs=2))
        qp = ctx.enter_context(tc.tile_pool(name="qp", bufs=1))
        kp = ctx.enter_context(tc.tile_pool(name="kp", bufs=1))
        vts = ctx.enter_context(tc.tile_pool(name="vts", bufs=2))
        es = ctx.enter_context(tc.tile_pool(name="es", bufs=2))
        aos = ctx.enter_context(tc.tile_pool(name="aos", bufs=2))
        rrp = ctx.enter_context(tc.tile_pool(name="rrp", bufs=2))
        rbp = ctx.enter_context(tc.tile_pool(name="rbp", bufs=2))
        tsp = ctx.enter_context(tc.tile_pool(name="tsp", bufs=2))
        h1p = ctx.enter_context(tc.tile_pool(name="h1p", bufs=2))

        ps_mm = ctx.enter_context(tc.tile_pool(name="ps_mm", bufs=2, space="PSUM"))
        ps_at = ctx.enter_context(tc.tile_pool(name="ps_at", bufs=2, space="PSUM"))
        ps_vt = ctx.enter_context(tc.tile_pool(name="ps_vt", bufs=1, space="PSUM"))
        ps_ao = ctx.enter_context(tc.tile_pool(name="ps_ao", bufs=1, space="PSUM"))
        ps_ob = ctx.enter_context(tc.tile_pool(name="ps_ob", bufs=2, space="PSUM"))

        units = [(b, n0, nn) for b in range(BPC)
                 for n0, nn in ((0, NN1), (NN1, NN2))] * repeat
        for b, n0, nn in units:
                padc = 2 + nn * PW
                x_t = xp.tile([C + 1, PADC_MAX], FR, tag="x")
                xf = x_t.bitcast(FP)
                # zero the 2 leading cols + inter-bn gaps; ones row
                nc.gpsimd.memset(xf[0:C, 0:2], 0.0)
                x3z = xf[0:C, 2:padc].rearrange("p (n t) -> p n t", t=PW)
                nc.gpsimd.memset(x3z[:, 0:nn, T:PW], 0.0)
                nc.gpsimd.memset(xf[C:C + 1, 0:padc], 1.0)
                # strided DMA in: tokens at stride 98
                x3 = x_t[0:C, 2:padc].rearrange("p (n t) -> p n t", t=PW)
                src = xin[b * C:(b + 1) * C, n0 * T:(n0 + nn) * T] \
                    .rearrange("p (n t) -> p n t", t=T)
                nc.sync.dma_start(out=x3[:, 0:nn, 0:T], in_=src)

                q_t = qp.tile([H, PADC_MAX], BF, tag="q")
                k_t = kp.tile([H, PADC_MAX], BF, tag="k")

                # Phase A: q|k via 3 accumulated tap matmuls per chunk.
                # fp32r needs 32-multiple moving width; remainder chunks
                # fall back to fp32 (bitcast).
                for c0 in range(2, padc, ACH):
                    cw = min(ACH, padc - c0)

                    def mop(ap):
                        return ap if cw % 32 == 0 else ap.bitcast(FP)
                    pa = ps_mm.tile([C, ACH], FP, tag="mm")
                    nc.tensor.matmul(pa[:, :cw], mop(wt2_sb[:]),
                                     mop(x_t[0:C, c0:c0 + cw]),
                                     start=True, stop=False)
                    nc.tensor.matmul(pa[:, :cw], mop(wt1_sb[:]),
                                     mop(x_t[0:C, c0 - 1:c0 - 1 + cw]),
                                     start=False, stop=False)
                    nc.tensor.matmul(pa[:, :cw], mop(wt0_sb[:]),
                                     mop(x_t[0:C, c0 - 2:c0 - 2 + cw]),
                                     start=False, stop=True)
                    nc.scalar.copy(out=q_t[:, c0:c0 + cw], in_=pa[0:H, :cw])
                    nc.vector.tensor_copy(k_t[:, c0:c0 + cw],
                                          pa[H:2 * H, :cw])

                # Phase B: attention in chunks of 5 bns
                for j0 in range(0, nn, GB):
                    g = min(GB, nn - j0)
                    gt = g * T

                    # vwt is zero-padded to 64 cols so the moving width is a
                    # 32-multiple (fp32r requirement); cols 0-31 = v, 32 = 1s
                    p_vt = ps_vt.tile([T, GB * 2 * H], FP, tag="vt")
                    for i in range(g):
                        cj = 2 + (j0 + i) * PW
                        nc.tensor.matmul(
                            p_vt[:, i * 2 * H:(i + 1) * 2 * H],
                            x_t[0:C + 1, cj:cj + T], vwt_sb[:],
                            start=True, stop=True, skip_group_check=True)
                    vt_sb = vts.tile([T, GB * (H + 1)], BF, tag="vt_sb")
                    nc.vector.tensor_copy(
                        vt_sb[:, :g * (H + 1)].rearrange(
                            "p (n c) -> p n c", c=H + 1),
                        p_vt[:, :g * 2 * H].rearrange(
                            "p (n c) -> p n c", c=2 * H)[:, :, 0:H + 1])

                    p_at = ps_at.tile([T, GB * T], FP, tag="at")
                    for i in range(g):
                        cj = 2 + (j0 + i) * PW
                        nc.tensor.matmul(p_at[:, i * T:(i + 1) * T],
                                         k_t[:, cj:cj + T],
                                         q_t[:, cj:cj + T],
                                         start=True, stop=True,
                                         skip_group_check=True)
                    e_sb = es.tile([T, GB * T], BF, tag="e")
                    nc.scalar.activation(out=e_sb[:, :gt], in_=p_at[:, :gt],
                                         func=ACT.Exp)
                    nc.gpsimd.tensor_mul(e_sb[:, :gt], e_sb[:, :gt],
                                         mask_sb[:, :gt])

                    p_ao = ps_ao.tile([H + 1, GB * T], FP, tag="ao")
                    for i in range(g):
                        nc.tensor.matmul(
                            p_ao[:, i * T:(i + 1) * T],
                            vt_sb[:, i * (H + 1):(i + 1) * (H + 1)],
                            e_sb[:, i * T:(i + 1) * T],
                            start=True, stop=True, skip_group_check=True)
                    ao_sb = aos.tile([H + 1, GB * T], BF, tag="ao_sb")
                    nc.scalar.copy(out=ao_sb[:, :gt], in_=p_ao[:, :gt])

                    r_t = rrp.tile([1, GB * T], BF, tag="r")
                    with nc.allow_low_precision(
                            reason="bf16 feed for PE 1/denom broadcast"):
                        nc.vector.reciprocal(
                            out=r_t[0:1, :gt],
                            in_=ao_sb[H:H + 1, :gt])

                    p_op = ps_ob.tile([C, GB * T], FP, tag="ob")
                    nc.tensor.matmul(p_op[:, :gt], owt_sb[:],
                                     ao_sb[0:H, :gt],
                                     start=True, stop=True)
                    p_rb = ps_ob.tile([C, GB * T], FP, tag="ob")
                    nc.tensor.matmul(p_rb[:, :gt], ones_sb[:],
                                     r_t[0:1, :gt],
                                     start=True, stop=True,
                                     skip_group_check=True)
                    rb_sb = rbp.tile([C, GB * T], BF, tag="rb")
                    nc.scalar.copy(out=rb_sb[:, :gt], in_=p_rb[:, :gt])
                    t_sb = tsp.tile([C, GB * T], BF, tag="t")
                    nc.vector.tensor_mul(t_sb[:, :gt], p_op[:, :gt],
                                         rb_sb[:, :gt])
                    # residual add into x_pad real slices
                    cj0 = 2 + j0 * PW
                    xv = xf[0:C, cj0:cj0 + g * PW] \
                        .rearrange("p (n t) -> p n t", t=PW)
                    xvr = x_t[0:C, cj0:cj0 + g * PW] \
                        .rearrange("p (n t) -> p n t", t=PW)
                    t3 = t_sb[:, :gt].rearrange("p (n t) -> p n t", t=T)
                    nc.vector.tensor_add(xvr[:, 0:g, 0:T], t3, xv[:, 0:g, 0:T])

                # Phase C: FFN per chunk, in place on x_pad
                for c0 in range(2, padc, ACH):
                    cw = min(ACH, padc - c0)

                    def mop(ap):
                        return ap if cw % 32 == 0 else ap.bitcast(FP)
                    p1 = ps_mm.tile([C, ACH], FP, tag="mm")
                    nc.tensor.matmul(p1[:, :cw], mop(ff1t_sb[:]),
                                     mop(x_t[0:C, c0:c0 + cw]),
                                     start=True, stop=True)
                    h1_sb = h1p.tile([C, ACH], BF, tag="h1")
                    nc.scalar.activation(out=h1_sb[:, :cw], in_=p1[:, :cw],
                                         func=ACT.Relu,
                                         bias=b1_sb[:, 0:1], scale=1.0)
                    p2 = ps_mm.tile([C, ACH], FP, tag="mm")
                    nc.tensor.matmul(p2[:, :cw], ff2t_sb[:],
                                     h1_sb[:, :cw],
                                     start=True, stop=True)
                    nc.vector.tensor_add(x_t[0:C, c0:c0 + cw], p2[:, :cw],
                                         xf[0:C, c0:c0 + cw])

                # DMA out (strided: skip gaps)
                y3 = yout[b * C:(b + 1) * C, n0 * T:(n0 + nn) * T] \
                    .rearrange("p (n t) -> p n t", t=T)
                xo = xf[0:C, 2:padc].rearrange("p (n t) -> p n t", t=PW)
                nc.sync.dma_start(out=y3, in_=xo[:, 0:nn, 0:T])

    return nc


def _prep_consts(q_w, k_w, v_w, o_w, ff_w1, ff_b1, ff_w2):
    f = np.float32
    # wt rows: [tap2(center); tap1; tap0], each [c, (q|k)]
    taps = []
    for kk in (2, 1, 0):
        taps.append(np.concatenate([q_w[:, :, kk], k_w[:, :, kk]], 0).T)
    wt = np.ascontiguousarray(np.concatenate(taps, 0), dtype=f)
    vwt = np.zeros((C + 1, 2 * H), f)
    vwt[0:C, 0:H] = v_w.T
    vwt[C, H] = 1.0
    ff1t = np.ascontiguousarray(ff_w1.T, dtype=f)
    b1 = np.ascontiguousarray(ff_b1.reshape(C, 1), dtype=f)
    import ml_dtypes
    bf = ml_dtypes.bfloat16
    mask1 = (np.arange(T)[:, None] <= np.arange(T)[None, :]).astype(bf)
    mask = np.ascontiguousarray(np.tile(mask1, (1, GB)))
    ff2t = np.ascontiguousarray(ff_w2.T.astype(bf))
    owt = np.ascontiguousarray(o_w.T.astype(bf))
    ones1 = np.ones((1, C), bf)
    return dict(wt=wt, vwt=vwt, owt=owt, ff1t=ff1t, ff2t=ff2t, b1=b1,
                mask=mask, ones1=ones1)


def kernel(x, q_w, k_w, v_w, o_w, ff_w1, ff_b1, ff_w2, ff_b2):
    from concourse.bass_utils import run_bass_kernel_spmd

    if "nc" not in _CACHE:
        _CACHE["nc"] = _build_program()
    nc = _CACHE["nc"]

    consts = _prep_consts(q_w, k_w, v_w, o_w, ff_w1, ff_b1, ff_w2)
    x = np.ascontiguousarray(x, dtype=np.float32)
    xv = x.reshape(NCORES, BPC * C, NT)

    in_maps = [{"xin": xv[i], **consts} for i in range(NCORES)]

    try:
        res = run_bass_kernel_spmd(nc, in_maps, list(range(NCORES)))
    except Exception:
        # a previously wedged device typically clears on retry
        res = run_bass_kernel_spmd(nc, in_maps, list(range(NCORES)))

    out = np.empty((B, C, N, T), np.float32)
    ov = out.reshape(NCORES, BPC * C, NT)
    for i in range(NCORES):
        ov[i] = res.results[i]["yout"]
    ff_b2 = np.asarray(ff_b2, np.float32)
    if ff_b2.any():
        out += ff_b2[None, :, None, None]
    return out
